# revision 1
# baseline (speedup 1.0000x reference)
"""AgentAttention Trainium2 kernel.

Full inputs -> shard batch over 8 NeuronCores (4 batches each, all params
replicated) -> bass/Tile kernel per core -> gather.

Per-core structure (software-pipelined over the 4 local batches):
  - x loaded in 112-row chunks (fp32), transposed on the PE (fp32 transpose,
    ACT drains cast to bf16) into x^T (c-part, n-free).
  - q^T/k^T via output-transposed GEMMs (head pairs land on 128-partition
    groups); v in natural layout (+ a ones column for stage-1 row sums) and
    PE-transposed into a zero-padded (30x32) bf16 image buffer for the dwc.
  - agent tokens = 4x4 avg pool of q', as two DVE reduces on q'^T; stored as
    block-diagonal (head-pair) tiles for one-matmul-per-pair attention.
  - the bias pipeline (bilinear 7->28 upsample + row/col broadcasts) is
    linear: folded into one constant Mcat (128,784); exp(bias) tables E1/E2
    are precomputed once so U = exp(scores)*E via one exp + one bf16 mul.
  - softmax without max-subtraction (scores are O(0.1) by construction);
    stage-1 normalization via a ones column in the v tiles + reciprocal on
    the tiny agent_v; stage-2 via block-ones matmul column sums, fast
    reciprocal, and a selector-matmul broadcast of 1/Z.
  - depthwise 3x3 conv = 9 per-partition-scalar MAC taps on DVE over the
    padded image, emitted at the end of each batch so it fills the DVE
    stream while the next batch's GEMMs run on PE.
  - each batch's normalize+proj tail is deferred past the next batch's GEMM
    phase (engine streams execute in order, so emission order is placement).
"""

import os
from contextlib import ExitStack

import ml_dtypes
import numpy as np

import concourse.bass as bass
import concourse.tile as tile
from concourse import bacc
from concourse import mybir
from concourse.bass_utils import run_bass_kernel_spmd
from concourse.masks import make_identity

F32 = mybir.dt.float32
BF16 = mybir.dt.bfloat16
ALU = mybir.AluOpType
AFT = mybir.ActivationFunctionType
AX = mybir.AxisListType

NCORES = 8
B, N, DIM, NH = 32, 784, 512, 8
BL = B // NCORES          # 4 batches per core
H = W = 28
HD = DIM // NH            # 64
AGENT = 49
POOL7 = 7
SCALE = HD ** -0.5        # 0.125
NCH = 7                   # 112-row chunks per image
CH = 112                  # chunk rows (= 4 image rows)
PW = 32                   # padded image row width (28 + pad, 64B aligned rows)
PH = 30                   # padded image height


def _u28() -> np.ndarray:
    """28x7 half-pixel bilinear upsample matrix (jax.image.resize bilinear)."""
    u = np.zeros((28, 7), np.float64)
    for y in range(28):
        t = (y + 0.5) * (7.0 / 28.0) - 0.5
        i0 = int(np.floor(t))
        w = t - i0
        a0 = min(max(i0, 0), 6)
        a1 = min(max(i0 + 1, 0), 6)
        u[y, a0] += 1.0 - w
        u[y, a1] += w
    return u.astype(np.float32)


def _mcat() -> np.ndarray:
    """(128, 784): rows 0:49 bilinear-up, 64:92 row-bcast, 96:124 col-bcast."""
    u = _u28()
    m = np.zeros((128, 784), np.float32)
    m[0:49] = np.kron(u, u).T
    m[64:92] = np.kron(np.eye(28, dtype=np.float32),
                       np.ones((1, 28), np.float32))
    m[96:124] = np.kron(np.ones((1, 28), np.float32),
                        np.eye(28, dtype=np.float32))
    return m



def _t(pool, shape, dtype, nm):
    return pool.tile(shape, dtype, name=nm, tag=nm)

def build_nc() -> bass.Bass:
    nc = bacc.Bacc("TRN2", target_bir_lowering=False, debug=False)

    x_d = nc.dram_tensor("x", [BL, N, DIM], F32, kind="ExternalInput").ap()
    qw_d = nc.dram_tensor("qw", [DIM, DIM], F32, kind="ExternalInput").ap()
    kvw_d = nc.dram_tensor("kvw", [2 * DIM, DIM], F32, kind="ExternalInput").ap()
    pw_d = nc.dram_tensor("pw", [DIM, DIM], F32, kind="ExternalInput").ap()
    pb_d = nc.dram_tensor("pb", [1, DIM], F32, kind="ExternalInput").ap()
    w9_d = nc.dram_tensor("w9", [DIM, 9], F32, kind="ExternalInput").ap()
    an_d = nc.dram_tensor("an", [NH, AGENT, 49], F32, kind="ExternalInput").ap()
    na_d = nc.dram_tensor("na", [NH, AGENT, 49], F32, kind="ExternalInput").ap()
    ah_d = nc.dram_tensor("ah", [NH, AGENT, 28], F32, kind="ExternalInput").ap()
    aw_d = nc.dram_tensor("aw", [NH, AGENT, 28], F32, kind="ExternalInput").ap()
    ha_d = nc.dram_tensor("ha", [NH, 28, AGENT], F32, kind="ExternalInput").ap()
    wa_d = nc.dram_tensor("wa", [NH, 28, AGENT], F32, kind="ExternalInput").ap()
    mcat_d = nc.dram_tensor("mcat", [128, N], BF16, kind="ExternalInput").ap()
    out_d = nc.dram_tensor("out", [BL, N, DIM], F32, kind="ExternalOutput").ap()

    with tile.TileContext(nc) as tc:
        with ExitStack() as ctx:
            _emit(ctx, tc, nc, locals())
    nc.compile()
    return nc


def _emit(ctx: ExitStack, tc: tile.TileContext, nc: bass.Bass, d: dict):
    x_d, qw_d, kvw_d, pw_d, pb_d, w9_d = (
        d["x_d"], d["qw_d"], d["kvw_d"], d["pw_d"], d["pb_d"], d["w9_d"])
    an_d, na_d, ah_d, aw_d, ha_d, wa_d = (
        d["an_d"], d["na_d"], d["ah_d"], d["aw_d"], d["ha_d"], d["wa_d"])
    mcat_d, out_d = d["mcat_d"], d["out_d"]

    singles = ctx.enter_context(tc.tile_pool(name="singles", bufs=1))

    # ---------------- constants / identities ----------------
    id_bf = _t(singles, [128, 128], BF16, "id_bf")
    make_identity(nc, id_bf)
    id_f32 = _t(singles, [128, 128], F32, "id_f32")
    make_identity(nc, id_f32)

    mcat = _t(singles, [128, N], BF16, "mcat")
    nc.sync.dma_start(out=mcat, in_=mcat_d)

    ones1 = _t(singles, [1, CH], BF16, "ones1")
    nc.vector.memset(ones1, 1.0)

    onesblk = _t(singles, [128, 2], BF16, "onesblk")
    nc.vector.memset(onesblk, 0.0)
    nc.vector.memset(onesblk[0:49, 0:1], 1.0)
    nc.vector.memset(onesblk[64:113, 1:2], 1.0)

    # sel8[hp]: lhsT with row 32hp -> ones in cols 0:64, row 32hp+1 -> cols
    # 64:128 (built transposed then PE-transposed, to keep partition starts
    # 32-aligned)
    sel8 = [_t(singles, [128, 128], BF16, f"sel8_{hp}") for hp in range(4)]

    projb = _t(singles, [1, DIM], BF16, "projb")
    w9 = [_t(singles, [128, 9], F32, f"w9_{g}") for g in range(4)]
    for g in range(4):
        nc.sync.dma_start(out=w9[g], in_=w9_d[g * 128:(g + 1) * 128, :])

    # weights W^T (c-part, o-free) bf16; o: [q 0:512|k 512:1024|v 1024:1536|proj 1536:2048]
    wT = [_t(singles, [128, 2048], BF16, f"wT{g}") for g in range(4)]
    bias1 = _t(singles, [128, 512], BF16, "bias1")  # an/ah/aw
    bias2 = _t(singles, [128, 512], BF16, "bias2")  # na/ha/wa
    nc.vector.memset(bias1, 0.0)
    nc.vector.memset(bias2, 0.0)

    with tc.tile_pool(name="prep", bufs=3) as tmp, \
         tc.tile_pool(name="ps_misc", bufs=2, space="PSUM") as ps_misc:
        pbs = _t(tmp, [1, DIM], F32, "pbs")
        nc.sync.dma_start(out=pbs, in_=pb_d)
        nc.vector.tensor_copy(projb, pbs)

        # v weights first (v-GEMM is the first consumer), then q, then k;
        # proj weights + bias tables are deferred into the pipeline below
        wtiles = [(kvw_d, t, 512 + t * 128) for t in range(4, 8)]
        wtiles += [(qw_d, t, t * 128) for t in range(4)]
        wtiles += [(kvw_d, t, 512 + t * 128) for t in range(4)]
        for src, t, ocol in wtiles:
            wnat = _t(tmp, [128, DIM], F32, "wnat")
            nc.sync.dma_start(out=wnat, in_=src[t * 128:(t + 1) * 128, :])
            for g in range(4):
                wps = _t(ps_misc, [128, 128], F32, "wps")
                nc.tensor.transpose(wps, wnat[:, g * 128:(g + 1) * 128],
                                    id_f32)
                nc.scalar.copy(out=wT[g][:, ocol:ocol + 128], in_=wps)
        for g in range(4):
            nc.vector.tensor_scalar(wT[g][:, 0:512], wT[g][:, 0:512], SCALE,
                                    None, ALU.mult)

    # ---------------- exp(bias) tables (once per core) ----------------
    # U = exp(scores + bias) = exp(scores) * exp(bias); E1/E2 hold exp(bias).
    e1 = _t(singles, [CH, NCH, 512], BF16, "e1")
    e2 = [_t(singles, [128, N], BF16, f"e2_{hp}") for hp in range(4)]

    # ---------------- per-batch pipeline ----------------
    # one shared rotating PSUM pool: 4 slots x 2 banks = all 8 banks
    ps = ctx.enter_context(tc.tile_pool(name="ps", bufs=4, space="PSUM"))
    ps2 = ctx.enter_context(tc.tile_pool(name="ps2", bufs=2, space="PSUM"))

    p_x = ctx.enter_context(tc.tile_pool(name="p_x", bufs=3))
    p_xT = ctx.enter_context(tc.tile_pool(name="p_xT", bufs=2))
    p_qk = ctx.enter_context(tc.tile_pool(name="p_qk", bufs=2))
    p_v = ctx.enter_context(tc.tile_pool(name="p_v", bufs=2))
    p_blk = ctx.enter_context(tc.tile_pool(name="p_blk", bufs=2))
    p_u = ctx.enter_context(tc.tile_pool(name="p_u", bufs=3))
    p_acc = ctx.enter_context(tc.tile_pool(name="p_acc", bufs=2))
    p_sm = ctx.enter_context(tc.tile_pool(name="p_sm", bufs=4))
    p_out = ctx.enter_context(tc.tile_pool(name="p_out", bufs=2))

    def emit_deferred_prologue():
        # proj weights (first used by tail(0), one batch later)
        for t in range(4):
            wnat = p_x.tile([128, DIM], F32, name="wnat", tag="xf", bufs=NCH + 1)
            nc.sync.dma_start(out=wnat, in_=pw_d[t * 128:(t + 1) * 128, :])
            for g in range(4):
                wps = _t(ps, [128, 128], F32, "ps")
                nc.tensor.transpose(wps, wnat[:, g * 128:(g + 1) * 128],
                                    id_f32)
                nc.scalar.copy(out=wT[g][:, 1536 + t * 128:1664 + t * 128],
                               in_=wps)

    def emit_bias_prologue():
        # bias concat assembly: per head h -> col block hp*128 + (h%2)*64
        for bsrc, dst, row0, ncol in [
            (an_d, bias1, 0, 49), (na_d, bias2, 0, 49),
            (ah_d, bias1, 64, 28), (aw_d, bias1, 96, 28),
        ]:
            for h in range(NH):
                cb = (h // 2) * 128 + (h % 2) * 64
                stg = p_x.tile([AGENT, ncol], F32, name="biasstg", tag="xf", bufs=NCH + 1)
                nc.sync.dma_start(out=stg, in_=bsrc[h])
                pt = _t(ps, [ncol, AGENT], F32, "ps")
                nc.tensor.transpose(pt, stg, id_f32[0:AGENT, 0:AGENT])
                nc.scalar.copy(out=dst[row0:row0 + ncol, cb:cb + 49], in_=pt)
        for bsrc, dst, row0 in [(ha_d, bias2, 64), (wa_d, bias2, 96)]:
            for h in range(NH):
                cb = (h // 2) * 128 + (h % 2) * 64
                stg = p_x.tile([28, AGENT], F32, name="hawastg", tag="xf", bufs=NCH + 1)
                nc.sync.dma_start(out=stg, in_=bsrc[h])
                nc.vector.tensor_copy(dst[row0:row0 + 28, cb:cb + 49], stg)
        # sel8 build (transposed write, PE transpose back)
        for hp in range(4):
            st = p_x.tile([128, 128], BF16, name="selstg", tag="xf", bufs=NCH + 1)
            nc.vector.memset(st, 0.0)
            nc.vector.memset(st[0:64, 32 * hp:32 * hp + 1], 1.0)
            nc.vector.memset(st[64:128, 32 * hp + 1:32 * hp + 2], 1.0)
            pt = _t(ps, [128, 128], BF16, "ps")
            nc.tensor.transpose(pt, st, id_bf)
            nc.scalar.copy(out=sel8[hp], in_=pt)
        # exp(bias) tables
        for c in range(NCH):
            pe1 = _t(ps, [CH, 512], F32, "ps")
            nc.tensor.matmul(pe1, lhsT=mcat[:, c * CH:(c + 1) * CH],
                             rhs=bias1, start=True, stop=True)
            nc.scalar.activation(e1[:, c, :], pe1, AFT.Exp)
        for hp in range(4):
            pe2 = _t(ps2, [128, N], F32, "ps2")
            for n0, nn in ((0, 512), (512, 272)):
                nc.tensor.matmul(pe2[:, n0:n0 + nn],
                                 lhsT=bias2[:, hp * 128:(hp + 1) * 128],
                                 rhs=mcat[:, n0:n0 + nn],
                                 start=True, stop=True)
            nc.scalar.activation(e2[hp], pe2, AFT.Exp)

    # dwc taps (dy, dx01): col offset 1+dx; dx==1 is 4B aligned (DVE 2x)
    taps_dve = [(0, 1), (1, 1), (2, 1), (1, 0), (0, 0), (2, 0), (0, 2),
                (1, 2), (2, 2)]

    emit_bias_prologue()
    pending_tail = [emit_deferred_prologue]
    for b in range(BL):
        # ---- load x(b), cast, transpose ----
        xT = [_t(p_xT, [128, N], BF16, f"xT{g}") for g in range(4)]
        xfs = []
        for c in range(NCH):
            xf = p_x.tile([CH, DIM], F32, name="xf", tag="xf", bufs=NCH + 1)
            nc.sync.dma_start(out=xf, in_=x_d[b, c * CH:(c + 1) * CH, :])
            xfs.append(xf)
        for g in range(4):
            for c0, nch in ((0, 4), (4, 3)):
                xps = _t(ps, [128, 4 * CH], F32, "ps")
                for j in range(nch):
                    nc.tensor.transpose(
                        xps[:, j * CH:(j + 1) * CH],
                        xfs[c0 + j][:, g * 128:(g + 1) * 128],
                        id_f32[0:CH, 0:CH])
                nc.scalar.copy(
                    out=xT[g][:, c0 * CH:(c0 + nch) * CH],
                    in_=xps[:, 0:nch * CH])

        # ---- v natural GEMM (+ones col) and padded v^T image ----
        vno = _t(p_v, [CH, NCH, NH, HD + 1], BF16, "vno")
        nc.gpsimd.memset(vno[:, :, :, HD:HD + 1], 1.0)
        vnat = _t(p_v, [CH, NCH, DIM], BF16, "vnat")
        vpad = [_t(p_v, [128, PH, PW], BF16, f"vpad{g}") for g in range(4)]
        for g in range(4):
            nc.gpsimd.memset(vpad[g], 0.0)
        for c in range(NCH):
            pv = _t(ps, [CH, DIM], F32, "ps")
            for g in range(4):
                nc.tensor.matmul(pv, lhsT=xT[g][:, c * CH:(c + 1) * CH],
                                 rhs=wT[g][:, 1024:1536],
                                 start=(g == 0), stop=(g == 3))
            nc.scalar.copy(out=vnat[:, c, :], in_=pv)
            nc.sync.dma_start(out=vno[:, c, :, 0:HD],
                              in_=vnat[:, c, :].rearrange("p (h d) -> p h d",
                                                          h=NH))
        for g2 in range(4):
            for c0, nch in ((0, 4), (4, 3)):
                pt = _t(ps, [128, 4 * CH], BF16, "ps")
                for j in range(nch):
                    nc.tensor.transpose(
                        pt[:, j * CH:(j + 1) * CH],
                        vnat[:, c0 + j, g2 * 128:(g2 + 1) * 128],
                        id_bf[0:CH, 0:CH])
                nc.scalar.copy(
                    out=vpad[g2][:, 1 + 4 * c0:1 + 4 * c0 + 4 * nch, 2:30],
                    in_=pt[:, 0:nch * CH].rearrange("p (y x) -> p y x", y=4 * nch))

        # ---- q^T, k^T output-transposed GEMMs ----
        qT = [_t(p_qk, [128, N], BF16, f"qT{g}") for g in range(4)]
        kT = [_t(p_qk, [128, N], BF16, f"kT{g}") for g in range(4)]
        for og in range(8):
            dst = qT[og] if og < 4 else kT[og - 4]
            for n0, nn in ((0, 512), (512, 272)):
                pq = _t(ps, [128, 512], F32, "ps")
                for g in range(4):
                    nc.tensor.matmul(pq[:, 0:nn],
                                     lhsT=wT[g][:, og * 128:og * 128 + 128],
                                     rhs=xT[g][:, n0:n0 + nn],
                                     start=(g == 0), stop=(g == 3))
                nc.scalar.copy(out=dst[:, n0:n0 + nn], in_=pq[:, 0:nn])

        # previous batch's normalize+proj tail: emitted here so this batch's
        # GEMM work precedes it in the in-order engine streams (it waits on
        # the previous batch's dwc chain on DVE)
        while pending_tail:
            pending_tail.pop(0)()

        # ---- agent pooling (4x4 avg of q') + block-diagonal tiles ----
        blkA = [_t(p_blk, [128, 128], BF16, f"blkA{g}") for g in range(4)]
        blkB = [_t(p_blk, [128, 128], BF16, f"blkB{g}") for g in range(4)]
        blkV = [_t(p_blk, [128, 128], BF16, f"blkV{g}") for g in range(4)]
        for g in range(4):
            s1p = _t(p_sm, [128, 28, 7], F32, "s1p")
            nc.vector.tensor_reduce(
                out=s1p, in_=qT[g].rearrange("p (y q xi) -> p y q xi",
                                             y=28, q=7),
                axis=AX.X, op=ALU.add)
            poolq = _t(p_sm, [128, 7, 7], F32, "poolq")
            nc.vector.tensor_reduce(
                out=poolq, in_=s1p.rearrange("p (pp yi) q -> p pp q yi", pp=7),
                axis=AX.X, op=ALU.add)
            nc.gpsimd.memset(blkA[g], 0.0)
            nc.gpsimd.memset(blkB[g], 0.0)
            nc.gpsimd.memset(blkV[g], 0.0)
            for half in range(2):
                r0, c0 = 64 * half, 64 * half
                nc.vector.tensor_scalar(
                    blkA[g][r0:r0 + 64, c0:c0 + 49].rearrange(
                        "p (q r) -> p q r", q=7),
                    poolq[r0:r0 + 64], 1.0 / 16.0, None, ALU.mult)
                nc.vector.tensor_scalar(
                    blkB[g][r0:r0 + 64, c0:c0 + 49].rearrange(
                        "p (q r) -> p q r", q=7),
                    poolq[r0:r0 + 64], 1.0 / (16.0 * SCALE), None, ALU.mult)

        # ---- stage 1: agents attend to keys ----
        u1s = []
        for c in range(NCH):
            u1 = p_u.tile([CH, 512], BF16, name="u1", tag="u1", bufs=NCH + 1)
            s1 = _t(ps, [CH, 512], F32, "ps")
            for hp in range(4):
                nc.tensor.matmul(s1[:, hp * 128:(hp + 1) * 128],
                                 lhsT=kT[hp][:, c * CH:(c + 1) * CH],
                                 rhs=blkA[hp], start=True, stop=True)
            et = p_sm.tile([CH, 512], BF16, name="et", tag="et", bufs=2)
            nc.scalar.activation(et, s1, AFT.Exp)
            nc.vector.tensor_tensor(u1, et, e1[:, c, :], ALU.mult)
            u1s.append(u1)
        for h in range(NH):
            g, half = h // 2, h % 2
            acol = g * 128 + half * 64
            av = _t(ps, [49, HD + 1], F32, "ps")
            for c in range(NCH):
                nc.tensor.matmul(av, lhsT=u1s[c][:, acol:acol + 49],
                                 rhs=vno[:, c, h, :],
                                 start=(c == 0), stop=(c == NCH - 1))
            rz = _t(p_sm, [49, 1], F32, "rz")
            nc.vector.reciprocal(rz, av[:, HD:HD + 1])
            nc.vector.tensor_scalar(
                blkV[g][64 * half:64 * half + 49, 64 * half:64 * half + 64],
                av[:, 0:HD], rz, None, ALU.mult)

        # ---- stage 2 + dwc ----
        z2all = p_sm.tile([128, N], F32, name="z2all", tag="z2all", bufs=1)
        nc.gpsimd.memset(z2all, 1.0)
        u2s = []
        for hp in range(4):
            u2 = p_u.tile([128, N], BF16, name="u2", tag="u2", bufs=5)
            sc = _t(ps2, [128, N], F32, "ps2")
            for n0, nn in ((0, 512), (512, 272)):
                nc.tensor.matmul(sc[:, n0:n0 + nn], lhsT=blkB[hp],
                                 rhs=qT[hp][:, n0:n0 + nn],
                                 start=True, stop=True)
            et2 = p_sm.tile([128, N], BF16, name="et2", tag="et2", bufs=2)
            nc.scalar.activation(et2, sc, AFT.Exp)
            nc.vector.tensor_tensor(u2, et2, e2[hp], ALU.mult)
            z2p = _t(ps2, [2, N], F32, "ps2")
            for n0, nn in ((0, 512), (512, 272)):
                nc.tensor.matmul(z2p[:, n0:n0 + nn], lhsT=onesblk,
                                 rhs=u2[:, n0:n0 + nn], start=True, stop=True)
            nc.scalar.copy(out=z2all[32 * hp:32 * hp + 2, :], in_=z2p)
            u2s.append(u2)
        # depthwise conv: emitted last so it tail-fills the DVE stream while
        # the next batch's GEMM phase runs on PE
        dacc = [_t(p_acc, [128, N], BF16, f"dacc{g}") for g in range(4)]
        for g in range(4):
            dacc_im = dacc[g].rearrange("p (y x) -> p y x", y=28)
            for i, (dy, dx) in enumerate(taps_dve):
                srcv = vpad[g][:, dy:dy + 28, 1 + dx:1 + dx + 28]
                k = dy * 3 + dx
                if i == 0:
                    nc.vector.tensor_scalar(dacc_im, srcv, w9[g][:, k:k + 1],
                                            None, ALU.mult)
                else:
                    nc.vector.scalar_tensor_tensor(
                        dacc_im, srcv, w9[g][:, k:k + 1], dacc_im,
                        ALU.mult, ALU.add)

        def make_tail(b, z2all, u2s, blkV, dacc):
            def emit_tail():
                rz2f = p_sm.tile([128, N], F32, name="rz2f", tag="rz2f",
                                 bufs=1)
                rscr = p_sm.tile([128, N], F32, name="rscr", tag="rscr",
                                 bufs=1)
                nc.vector.reciprocal_approx_accurate(rz2f, z2all, rscr)
                rz2 = p_sm.tile([128, N], BF16, name="rz2", tag="rz2", bufs=2)
                nc.vector.tensor_copy(rz2, rz2f)
                accT = [_t(p_acc, [128, N], BF16, f"accT{g}")
                        for g in range(4)]
                for hp in range(4):
                    zb = _t(ps2, [128, N], F32, "ps2")
                    ot = _t(ps2, [128, N], F32, "ps2")
                    for n0, nn in ((0, 512), (512, 272)):
                        nc.tensor.matmul(zb[:, n0:n0 + nn], lhsT=sel8[hp],
                                         rhs=rz2[:, n0:n0 + nn],
                                         start=True, stop=True)
                        nc.tensor.matmul(ot[:, n0:n0 + nn], lhsT=blkV[hp],
                                         rhs=u2s[hp][:, n0:n0 + nn],
                                         start=True, stop=True)
                    zbs = p_sm.tile([128, N], BF16, name="zbs", tag="zbs",
                                    bufs=2)
                    nc.scalar.copy(out=zbs, in_=zb)
                    nc.vector.tensor_tensor(accT[hp], ot, zbs, ALU.mult)
                    nc.vector.tensor_tensor(accT[hp], accT[hp], dacc[hp],
                                            ALU.add)
                for c in range(NCH):
                    pp = _t(ps, [CH, DIM], F32, "ps")
                    for g in range(4):
                        nc.tensor.matmul(pp,
                                         lhsT=accT[g][:, c * CH:(c + 1) * CH],
                                         rhs=wT[g][:, 1536:2048],
                                         start=(g == 0), stop=False)
                    nc.tensor.matmul(pp, lhsT=ones1, rhs=projb,
                                     start=False, stop=True)
                    ob = _t(p_out, [CH, DIM], F32, "ob")
                    nc.scalar.copy(out=ob, in_=pp)
                    nc.sync.dma_start(out=out_d[b, c * CH:(c + 1) * CH, :],
                                      in_=ob)
            return emit_tail
        pending_tail.append(make_tail(b, z2all, u2s, blkV, dacc))

    for fn in pending_tail:
        fn()


_NC_CACHE = None


def _get_nc():
    global _NC_CACHE
    if _NC_CACHE is None:
        _NC_CACHE = build_nc()
    return _NC_CACHE


def make_in_maps(inputs: dict) -> list:
    """Shard + host-side reshapes (no compute beyond constant Mcat)."""
    f = lambda k: np.ascontiguousarray(np.asarray(inputs[k], dtype=np.float32))
    x = f("x")
    qw, kvw, pw = f("q_w"), f("kv_w"), f("proj_w")
    pb = f("proj_b").reshape(1, DIM)
    w9 = f("dwc_w").reshape(DIM, 9)
    dwc_b = f("dwc_b")
    assert np.abs(dwc_b).max() == 0.0 or True
    # dwc bias folds into proj bias: out = (attn + dwc + dwc_b) @ pw^T + pb
    #   -> pb_eff = pb + dwc_b @ pw^T
    pb = pb + dwc_b.reshape(1, DIM) @ pw.T
    an = f("an_bias").reshape(NH, AGENT, 49)
    na = f("na_bias").reshape(NH, AGENT, 49)
    ah = f("ah_bias").reshape(NH, AGENT, 28)
    aw = f("aw_bias").reshape(NH, AGENT, 28)
    ha = f("ha_bias").reshape(NH, 28, AGENT)
    wa = f("wa_bias").reshape(NH, 28, AGENT)
    mcat = _mcat().astype(ml_dtypes.bfloat16)
    shared = dict(qw=qw, kvw=kvw, pw=pw, pb=pb, w9=w9, an=an, na=na, ah=ah,
                  aw=aw, ha=ha, wa=wa, mcat=mcat)
    return [dict(x=np.ascontiguousarray(x[i * BL:(i + 1) * BL]), **shared)
            for i in range(NCORES)]


def kernel(**inputs) -> np.ndarray:
    nc = _get_nc()
    in_maps = make_in_maps(inputs)
    trace = os.environ.get("KERNEL_TRACE", "0") == "1"
    res = run_bass_kernel_spmd(nc, in_maps, core_ids=list(range(NCORES)),
                               trace=trace)
    if trace and res.exec_time_ns is not None:
        print(f"HW exec time: {res.exec_time_ns} ns")
        kernel.last_exec_time_ns = res.exec_time_ns
        kernel.last_trace = res.instructions_and_trace
    out = np.concatenate([r["out"] for r in res.results], axis=0)
    return np.ascontiguousarray(out.astype(np.float32))

